# revision 3
# baseline (speedup 1.0000x reference)
"""ContinuousDeepFM Trainium2 kernel (8-core data-parallel over batch).

Math (algebraically collapsed from the reference — the [B,D,D] interaction
tensor is never materialized):
    fo  = x @ W1 + bias
    xw  = x @ W2
    so[b,j] = 0.5 * xw[b,j]^2 * (sum_i x[b,i]^2 - (sum_i x[b,i])^2)
    h   = MLP(x @ Wf)   (3 ReLU layers + final linear, weights mlp_w[i].T)
    out = fo + so + h

Sharding: batch 512 -> 64 rows per core; all weights replicated.
On-chip layout is feature-major (activations stored transposed, [D, 64] as
4 chunks of 128 partitions) so no on-chip transposes are needed anywhere;
per-feature biases become per-partition scalars.

Precision: the output is dominated by the second-order term (RMS ~2e5 vs
~23 for fo and ~1 for h), so W2/x/so stay fp32 while W1/Wf/mlp_w run in
bf16 (measured end-to-end rel err ~1e-6, same as pure fp32, at half the
weight-DMA bytes).
"""

import numpy as np
import ml_dtypes

B = 512
D = 512
NCORES = 8
BL = B // NCORES  # 64 batch rows per core
P = 128
KC = D // P  # 4 partition chunks per feature dim

_NC_CACHE = {}


def _split_multi_waits(nc, mybir):
    """This container's walrus build supports only ONE sync wait per
    instruction, but Tile's scheduler attaches several (e.g. the exit
    drain). Split extras into preceding single-wait NoOps on the same
    engine — in-order execution preserves the barrier semantics."""
    ctr = 0
    for fn in nc.m.functions:
        for blk in fn.blocks:
            insts = blk.instructions
            if not any(
                i.sync_info is not None
                and i.sync_info.on_wait
                and len(i.sync_info.on_wait) > 1
                for i in insts
            ):
                continue
            out = []
            for inst in insts:
                si = inst.sync_info
                if si is not None and si.on_wait and len(si.on_wait) > 1:
                    waits = list(si.on_wait)
                    for w in waits[:-1]:
                        ctr += 1
                        nop = mybir.InstNoOp(
                            name=f"wsplit-{ctr}-{inst.name}", ins=[], outs=[]
                        )
                        nop.engine = inst.engine
                        nop.sync_info = mybir.SyncInfo(on_wait=[w], on_update=[])
                        out.append(nop)
                    si.on_wait = [waits[-1]]
                out.append(inst)
            blk.instructions = out
    return ctr


def _build_nc():
    import concourse.bass as bass
    import concourse.mybir as mybir
    import concourse.tile as tile

    dt = mybir.dt
    f32 = dt.float32
    bf16 = dt.bfloat16
    Alu = mybir.AluOpType

    nc = bass.Bass("TRN2", target_bir_lowering=False, debug=False)

    xT = nc.dram_tensor("xT", [D, BL], f32, kind="ExternalInput")
    w2 = nc.dram_tensor("w2", [D, D], f32, kind="ExternalInput")
    w1 = nc.dram_tensor("w1", [D, D], bf16, kind="ExternalInput")
    wf = nc.dram_tensor("wf", [D, D], bf16, kind="ExternalInput")
    mwT = nc.dram_tensor("mwT", [4, D, D], bf16, kind="ExternalInput")
    btot = nc.dram_tensor("btot", [D], f32, kind="ExternalInput")
    mb = nc.dram_tensor("mb", [3, D], f32, kind="ExternalInput")
    outT = nc.dram_tensor("outT", [D, BL], f32, kind="ExternalOutput")

    xT_r = xT.ap().rearrange("(c p) b -> p c b", p=P)
    w2_r = w2.ap().rearrange("(c p) j -> p c j", p=P)
    w1_r = w1.ap().rearrange("(c p) j -> p c j", p=P)
    wf_r = wf.ap().rearrange("(c p) j -> p c j", p=P)
    mwT_r = mwT.ap().rearrange("i (c p) j -> p i c j", p=P)
    btot_r = btot.ap().rearrange("(c p) -> p c", p=P)
    mb_r = mb.ap().rearrange("i (c p) -> p i c", p=P)
    outT_r = outT.ap().rearrange("(c p) b -> p c b", p=P)

    with tile.TileContext(nc) as tc:
        with (
            tc.tile_pool(name="w", bufs=1) as wpool,
            tc.tile_pool(name="act", bufs=1) as apool,
            tc.tile_pool(name="ps", bufs=1, space="PSUM") as pspool,
        ):
            # constants
            ones = apool.tile([P, 1], f32, tag="ones")
            nc.vector.memset(ones[:], 1.0)
            halfrow = apool.tile([1, P], f32, tag="halfrow")
            nc.vector.memset(halfrow[:], 0.5)

            # input x (feature-major) + derived tensors
            xt = apool.tile([P, KC, BL], f32, tag="xt")
            nc.sync.dma_start(xt[:], xT_r)
            btot_sb = apool.tile([P, KC], f32, tag="btot")
            nc.sync.dma_start(btot_sb[:], btot_r)
            mb_sb = apool.tile([P, 3, KC], f32, tag="mb")
            nc.sync.dma_start(mb_sb[:], mb_r)

            # weight loads, in consumption order (w2 -> wf -> mlp -> w1)
            w2_sb = wpool.tile([P, KC, D], f32, tag="w2")
            for kc in range(KC):
                nc.sync.dma_start(w2_sb[:, kc, :], w2_r[:, kc, :])
            wf_sb = wpool.tile([P, KC, D], bf16, tag="wf")
            for kc in range(KC):
                nc.sync.dma_start(wf_sb[:, kc, :], wf_r[:, kc, :])
            mw_sb = wpool.tile([P, 4, KC, D], bf16, tag="mw")
            for i in range(4):
                for kc in range(KC):
                    nc.sync.dma_start(mw_sb[:, i, kc, :], mwT_r[:, i, kc, :])
            w1_sb = wpool.tile([P, KC, D], bf16, tag="w1")
            for kc in range(KC):
                nc.sync.dma_start(w1_sb[:, kc, :], w1_r[:, kc, :])

            # xc = [x | x^2] interleaved per chunk (fp32); xtb = bf16 copy
            xc = apool.tile([P, KC, 2, BL], f32, tag="xc")
            nc.vector.tensor_copy(xc[:, :, 0, :], xt[:])
            nc.vector.tensor_mul(xc[:, :, 1, :], xt[:], xt[:])
            xtb = apool.tile([P, KC, BL], bf16, tag="xtb")
            nc.vector.tensor_copy(xtb[:], xt[:])

            # s1/s2 partition reduction via ones-matmul -> [1, 2, BL]
            s_ps = pspool.tile([1, 2, BL], f32, tag="s")
            for kc in range(KC):
                nc.tensor.matmul(
                    s_ps[:],
                    ones[:],
                    xc[:, kc, :, :],
                    start=(kc == 0),
                    stop=(kc == KC - 1),
                )
            s_sb = apool.tile([1, 2, BL], f32, tag="s_sb")
            nc.vector.tensor_copy(s_sb[:], s_ps[:])
            s1sq = apool.tile([1, BL], f32, tag="s1sq")
            nc.vector.tensor_mul(s1sq[:], s_sb[:, 0, :], s_sb[:, 0, :])
            t_sb = apool.tile([1, BL], f32, tag="t_sb")
            nc.vector.tensor_sub(t_sb[:], s_sb[:, 1, :], s1sq[:])

            # broadcast 0.5*t across partitions via K=1 matmul
            t128_ps = pspool.tile([P, BL], f32, tag="t128")
            nc.tensor.matmul(t128_ps[:], halfrow[:], t_sb[:], start=True, stop=True)
            t128_sb = apool.tile([P, BL], f32, tag="t128sb")
            nc.vector.tensor_copy(t128_sb[:], t128_ps[:])

            # xw = x @ W2 (fp32, feature-major) ; xwsq = xw^2 (ScalarE to
            # avoid a dual-PSUM-read on DVE)
            xw_ps = [pspool.tile([P, BL], f32, tag="mm", bufs=6, name=f"xw{j}") for j in range(KC)]
            for kc in range(KC):
                for jc in range(KC):
                    nc.tensor.matmul(
                        xw_ps[jc][:],
                        w2_sb[:, kc, jc * P : (jc + 1) * P],
                        xt[:, kc, :],
                        start=(kc == 0),
                        stop=(kc == KC - 1),
                    )
            xwsq = apool.tile([P, KC, BL], f32, tag="xwsq")
            for jc in range(KC):
                nc.scalar.square(xwsq[:, jc, :], xw_ps[jc][:])

            # so2 = 0.5 * xw^2 * t + btot   (btot = bias + mlp_b[3])
            so = apool.tile([P, KC, BL], f32, tag="so")
            for jc in range(KC):
                nc.vector.tensor_mul(so[:, jc, :], xwsq[:, jc, :], t128_sb[:])
            so2 = apool.tile([P, KC, BL], f32, tag="so2")
            for jc in range(KC):
                nc.vector.tensor_scalar(
                    so2[:, jc, :],
                    so[:, jc, :],
                    btot_sb[:, jc : jc + 1],
                    None,
                    op0=Alu.add,
                )

            # deep: h0 = x @ Wf (bf16), no activation
            h_ps = [pspool.tile([P, BL], f32, tag="mm", bufs=6, name=f"h0p{j}") for j in range(KC)]
            for kc in range(KC):
                for jc in range(KC):
                    nc.tensor.matmul(
                        h_ps[jc][:],
                        wf_sb[:, kc, jc * P : (jc + 1) * P],
                        xtb[:, kc, :],
                        start=(kc == 0),
                        stop=(kc == KC - 1),
                    )
            h = apool.tile([P, KC, BL], bf16, tag="h0")
            for jc in range(KC):
                nc.vector.tensor_copy(h[:, jc, :], h_ps[jc][:])

            # hidden layers 0..2: h = relu(h @ mw[i].T + mb[i])
            for i in range(3):
                l_ps = [pspool.tile([P, BL], f32, tag="mm", bufs=6, name=f"l{i}p{j}") for j in range(KC)]
                for kc in range(KC):
                    for jc in range(KC):
                        nc.tensor.matmul(
                            l_ps[jc][:],
                            mw_sb[:, i, kc, jc * P : (jc + 1) * P],
                            h[:, kc, :],
                            start=(kc == 0),
                            stop=(kc == KC - 1),
                        )
                hn = apool.tile([P, KC, BL], bf16, tag=f"h{i + 1}")
                for jc in range(KC):
                    nc.vector.tensor_scalar(
                        hn[:, jc, :],
                        l_ps[jc][:],
                        mb_sb[:, i, jc : jc + 1],
                        0.0,
                        op0=Alu.add,
                        op1=Alu.max,
                    )
                h = hn

            # final: o = h3 @ mw[3].T + x @ W1 accumulated into one psum,
            # then out = o + so2
            o_ps = [pspool.tile([P, BL], f32, tag="mm", bufs=6, name=f"op{j}") for j in range(KC)]
            for kc in range(KC):
                for jc in range(KC):
                    nc.tensor.matmul(
                        o_ps[jc][:],
                        mw_sb[:, 3, kc, jc * P : (jc + 1) * P],
                        h[:, kc, :],
                        start=(kc == 0),
                        stop=False,
                    )
            for kc in range(KC):
                for jc in range(KC):
                    nc.tensor.matmul(
                        o_ps[jc][:],
                        w1_sb[:, kc, jc * P : (jc + 1) * P],
                        xtb[:, kc, :],
                        start=False,
                        stop=(kc == KC - 1),
                    )
            out_sb = apool.tile([P, KC, BL], f32, tag="out")
            for jc in range(KC):
                nc.vector.tensor_add(out_sb[:, jc, :], o_ps[jc][:], so2[:, jc, :])

            nc.sync.dma_start(outT_r, out_sb[:])

    _split_multi_waits(nc, mybir)
    return nc


def _get_nc():
    if "nc" not in _NC_CACHE:
        _NC_CACHE["nc"] = _build_nc()
    return _NC_CACHE["nc"]


def kernel(**inputs):
    from concourse.bass_utils import run_bass_kernel_spmd

    x = np.asarray(inputs["x"], np.float32)
    w1 = np.asarray(inputs["first_order_weights"], np.float32)
    bias = np.asarray(inputs["bias"], np.float32)
    w2 = np.ascontiguousarray(np.asarray(inputs["second_order_weights"], np.float32))
    wf = np.asarray(inputs["feature_weights"], np.float32)
    mw = np.asarray(inputs["mlp_w"], np.float32)
    mb = np.asarray(inputs["mlp_b"], np.float32)

    bf16 = ml_dtypes.bfloat16
    xT = np.ascontiguousarray(x.T)
    w1b = np.ascontiguousarray(w1.astype(bf16))
    wfb = np.ascontiguousarray(wf.astype(bf16))
    mwT = np.ascontiguousarray(mw.transpose(0, 2, 1)).astype(bf16)
    btot = np.ascontiguousarray((bias + mb[3]).astype(np.float32))
    mb3 = np.ascontiguousarray(mb[:3].astype(np.float32))

    nc = _get_nc()
    in_maps = [
        {
            "xT": np.ascontiguousarray(xT[:, c * BL : (c + 1) * BL]),
            "w2": w2,
            "w1": w1b,
            "wf": wfb,
            "mwT": mwT,
            "btot": btot,
            "mb": mb3,
        }
        for c in range(NCORES)
    ]
    res = run_bass_kernel_spmd(nc, in_maps, core_ids=list(range(NCORES)))
    out = np.empty((B, D), np.float32)
    for c in range(NCORES):
        out[c * BL : (c + 1) * BL, :] = res.results[c]["outT"].T
    return out
